# revision 62
# baseline (speedup 1.0000x reference)
"""CondConv2d (MoE routed conv) Trainium2 kernel.

Math: out[b] = sum_e routing[b,e] * conv3x3(x[b], W[e])
Since the expert mix is linear in W, this equals
    out[b] = conv3x3(x[b], Wmix_b),  Wmix_b = sum_e routing[b,e] * W[e]
which needs 1 conv per sample instead of E=4 (4x less PE work).

Sharding: data-parallel over batch, B=16 -> 2 samples per core on 8 cores.
Weights (all 4 experts, transposed to [ci, tap, e, co] on host) are
replicated; the per-sample mix happens on-device on the Vector engine.

Conv as implicit GEMM: x is zero-padded on host to [ci, 58, 58]; for each
of 9 taps the matmul streams a shifted window of the padded image
(rhs = xpad[:, blk*8+kh : +8, kw : kw+56], N=448) against the tap's mixed
weight slice (lhsT = Wmix[ci, co], K=ci on partitions), accumulating all
9 taps into one PSUM bank (fp32). 7 row-blocks of 8 rows cover the 56
output rows.

Numerics: x and W are fp16 on the wire; matmuls run fp16 at 1 cycle/row
with fp32 PSUM accumulation (~4e-4 L2 rel err). The output is stored as
fp16 (upcast to fp32 on host), halving store traffic; the extra fp16
rounding adds ~3e-4, total ~5e-4 -- far inside the 2e-2 gate.

Schedule: the whole kernel is DMA-need-ordered. Loads are issued in strict
global need order, alternating between the sync and scalar DGE rings so
both rings' FIFOs drain in need order while sharing the ~330 GB/s HBM
bandwidth: wt tap0+routing first (gates the first weight mix), then the
four x0 row-chunks (tap-outer sample 0 sweeps the full image on its first
tap), with the per-tap weight chunks interleaved where they are needed,
then sample 1's x. Sample 0 runs tap-outer over 7 live PSUM banks;
sample 1 runs block-outer (9 taps into one bank, then drain) so its
output streams out incrementally. PSUM drains are split between the
Scalar (activation-copy) and Vector engines and convert fp32->fp16;
stores are batched into few large-line DMAs. Dummy matmuls on a zeroed
tile cover the initial load phase to keep the PE HAM clock-gate warm.
"""

import os
import sys

os.environ.setdefault("MYCRO_LOCAL_CACHE", "1")
for _p in ("/opt/trn_rl_repo",):
    if _p not in sys.path:
        sys.path.insert(0, _p)

import numpy as np

B, CIN, COUT, H, W_SP = 16, 128, 128, 56, 56
E, KH, KW = 4, 3, 3
NCORES = 8
SPC = B // NCORES          # samples per core
HP, WP = H + 2, W_SP + 2   # padded spatial
NTAP = KH * KW
RPB = 8                    # output rows per matmul block
NBLK = H // RPB
NT = RPB * W_SP            # moving-operand free size per matmul (448)
N_WARM = 7                 # big (N=512) HAM warm-up dummy matmuls
N_WARM_SM = 8              # small (N=128) bridge dummies before the stream
# The PE HAM clock needs ~4us of GAPLESS activity to ramp to 2.4 GHz and
# any idle resets it; the x/weight chase in sample-0's first taps has
# unavoidable DMA waits, so small dummy matmuls (N=128) pad those gaps.
# Keys are (tap, blk) after whose matmul the pads are emitted; sized to
# overshoot the expected wait (overshoot costs ~0.1us/dummy, a reset
# costs ~2us of half-rate stream).
# sample-0 blocks in x-arrival order: b6's chunk rides the otherwise-idle
# GpSimd ring and lands first, so it runs ahead of the late-arriving
# middle chunks (real work instead of pad during the x0c1 wait)
BLK_ORDER0 = [0, 1, 6, 2, 3, 4, 5]
# keyed by (tap, blk)
PADS = {(0, 1): 2, (0, 6): 14, (0, 3): 6, (0, 5): 4,
        (1, 5): 4, (2, 5): 4, (3, 5): 2}

# sample-0 x row chunks (start_row, n_rows) and block->chunk map; chunks
# overlap by 2 rows so each 8-row output block reads one chunk only
XCH0 = [(0, 18), (16, 18), (32, 18), (48, 10)]
BLK_CH0 = [0, 0, 1, 1, 2, 2, 3]
XCH1 = [(0, 34), (32, 26)]
# sample-1 row blocks (r0, nr, chunk); 1-row final block minimizes the
# last accumulate + copy + store in the kernel tail
BLKS1 = [(0, 8, 0), (8, 8, 0), (16, 8, 0), (24, 8, 0),
         (32, 8, 1), (40, 8, 1), (48, 7, 1), (55, 1, 1)]
# weight-tap DMA chunks (start_tap, n_taps), matching the mix chunks
WTCH = [(1, 2), (3, 2), (5, 2), (7, 2)]
# sample-0 mix chunks. Two-tap chunks amortize the 4-op chain to
# ~0.85us/tap — below the PE's 1.31us/tap consumption, so the mix
# pipeline has slack at every tap boundary. Each chunk gets its OWN tile
# (matmul weight reads are tracked whole-tile, so chunks sharing a tile
# would serialize behind earlier matmuls)
MIXCH = [(0, 1), (1, 2), (3, 2), (5, 2), (7, 2)]
# sample-1 mix chunks (vector runs them after sample 0's, well before use)
MIXCH1 = [(0, 3), (3, 3), (6, 3)]

_cached_nc = None


def _build_nc():
    import concourse.tile as tile
    from concourse import bacc, mybir

    f32 = mybir.dt.float32
    f16 = mybir.dt.float16
    MUL, ADD = mybir.AluOpType.mult, mybir.AluOpType.add

    nc = bacc.Bacc(
        "TRN2", target_bir_lowering=False, debug=False, num_devices=NCORES
    )

    xpad_d = nc.dram_tensor(
        "xpad", [SPC, CIN, HP * WP], f16, kind="ExternalInput"
    ).ap()
    # host layout: [ci, (rb | tap, e, co)] — routing scalars (fp32 bits
    # packed into 2 fp16 slots each; tensor_scalar wants fp32 scalars)
    # share the weight tensor so one DMA delivers both rb and tap 0
    TAPW = E * COUT          # 512 halfs per tap in wt
    RBW = SPC * E * 2        # fp32 scalars as fp16 slot pairs
    wt_d = nc.dram_tensor(
        "wt", [CIN, RBW + NTAP * TAPW], f16, kind="ExternalInput"
    ).ap()
    out_d = nc.dram_tensor(
        "out", [SPC, COUT, H * W_SP], f16, kind="ExternalOutput"
    ).ap()

    with tile.TileContext(nc) as tc:
        with (
            tc.tile_pool(name="const", bufs=1) as cst,
            tc.tile_pool(name="x", bufs=1) as xpool,
            tc.tile_pool(name="wmix", bufs=1) as wmp,
            tc.tile_pool(name="ob", bufs=2) as opool,
            tc.tile_pool(name="ps", bufs=8, space="PSUM") as pspool,
        ):
            # --- HAM warm-up: dummy matmuls on a zeroed tile during loads
            zt = cst.tile([128, 512], f16, tag="zero")
            nc.gpsimd.memset(zt[:], 0.0)
            warm_ps = pspool.tile([128, 512], f32, tag="ps")
            for _ in range(N_WARM):
                nc.tensor.matmul(
                    warm_ps[:], zt[:, :128], zt[:], start=True, stop=True
                )
            for _ in range(N_WARM_SM):
                nc.tensor.matmul(
                    warm_ps[:, :128], zt[:, :128], zt[:, :128],
                    start=True, stop=True,
                )

            wt_t = cst.tile([CIN, RBW + NTAP * TAPW], f16, tag="wt")
            rb_t = wt_t[:, 0:RBW].bitcast(f32)  # [128, SPC*E] fp32

            def load_wt_chunk(t0, ntaps, eng):
                # first chunk also carries the routing scalars
                lo = 0 if t0 == 0 else RBW + t0 * TAPW
                sl = slice(lo, RBW + (t0 + ntaps) * TAPW)
                eng.dma_start(wt_t[:, sl], wt_d[:, sl])

            def load_x_chunk(s, xtiles, xch, c, eng):
                r0, nr = xch[c]
                xt = xpool.tile([CIN, nr * WP], f16, tag=f"x{s}_{c}",
                                name=f"x{s}_{c}")
                sl = slice(r0 * WP, (r0 + nr) * WP)
                eng.dma_start(xt[:], xpad_d[s][:, sl])
                xtiles[c] = xt

            # Concurrent DGE queues share HBM bandwidth round-robin and
            # each engine's DMA_DIRECT2D costs ~0.7us of descriptor
            # generation, so the critical prefix (x0 + early weight taps)
            # is split across just the sync and scalar rings in strict
            # global need order, with few DMAs per ring. Sample-1's x
            # goes on the GpSimd ring but GATED behind x0's last chunk
            # (via a dummy tile read) so its transfers cannot steal
            # bandwidth from the critical window. Stores later reuse the
            # sync ring.
            x0t = [None] * len(XCH0)
            x1t = [None] * len(XCH1)
            load_wt_chunk(0, 1, nc.sync)           # rb + tap 0 weights
            load_x_chunk(0, x0t, XCH0, 0, nc.scalar)
            load_x_chunk(0, x0t, XCH0, 3, nc.gpsimd)
            load_wt_chunk(1, 2, nc.sync)
            load_x_chunk(0, x0t, XCH0, 1, nc.scalar)
            load_x_chunk(0, x0t, XCH0, 2, nc.sync)
            load_wt_chunk(3, 2, nc.scalar)
            load_wt_chunk(5, 2, nc.sync)
            load_wt_chunk(7, 2, nc.scalar)
            # gate: write into the x1 tiles from a copy that reads the
            # LAST weight chunk's region — the WAW overlap forces the x1
            # DMAs (which overwrite the same tiles) to wait until all
            # critical loads are done, so x1 transfers cannot steal
            # bandwidth from them. (A side-effect-free copy is NOT
            # enough: the tile scheduler reorders ops with no data dep.)
            gate_col = RBW + 8 * TAPW
            for c, (r0, nr) in enumerate(XCH1):
                xt = xpool.tile([CIN, nr * WP], f16, tag=f"x1_{c}",
                                name=f"x1_{c}")
                nc.gpsimd.tensor_copy(
                    xt[:, 0:1], wt_t[:, gate_col : gate_col + 1]
                )
                sl = slice(r0 * WP, (r0 + nr) * WP)
                nc.gpsimd.dma_start(xt[:], xpad_d[1][:, sl])
                x1t[c] = xt

            # warm the Activation engine's Copy table during the load
            # phase so the first real PSUM drain doesn't pay the load
            aw = cst.tile([128, 1], f16, tag="actwarm")
            nc.scalar.copy(aw[:], zt[:, 0:1])

            wt3 = wt_t[:, RBW:].rearrange("p (t e c) -> p t e c", t=NTAP, e=E)

            def mix(dst3, s, t0, t1, e_lo=0, e_hi=E):
                """dst3 = sum_{e in [e_lo,e_hi)} rb[s,e] * wt[:, t0:t1, e, :]"""
                first = True
                for e in range(e_lo, e_hi):
                    sc = rb_t[:, s * E + e : s * E + e + 1]
                    src = wt3[:, t0:t1, e, :]
                    if first:
                        nc.vector.tensor_scalar_mul(dst3, src, sc)
                        first = False
                    else:
                        nc.vector.scalar_tensor_tensor(
                            dst3, src, sc, dst3, MUL, ADD
                        )

            def mix_chunks(s, chlist, prefix):
                out = {}
                for c, (t0, ntc) in enumerate(chlist):
                    wmt = wmp.tile(
                        [CIN, ntc * COUT], f16, tag=f"{prefix}{c}",
                        name=f"{prefix}{c}",
                    )
                    wm3 = wmt.rearrange("p (t c) -> p t c", t=ntc)
                    mix(wm3, s, t0, t0 + ntc)
                    for tt in range(t0, t0 + ntc):
                        out[tt] = (wmt, tt - t0)
                return out

            wm0 = mix_chunks(0, MIXCH, "wm0_")
            wm1 = mix_chunks(1, MIXCH1, "wm1_")

            def rhs_ap(xtiles, c, r0, nr, kh, kw):
                xch = XCH0 if xtiles is x0t else XCH1
                loc = r0 - xch[c][0]
                x3 = xtiles[c][:].rearrange("p (h w) -> p h w", w=WP)
                return x3[:, loc + kh : loc + kh + nr, kw : kw + W_SP]

            def copy_block(eng, ob, ps, r0, nr):
                sl = slice(r0 * W_SP, (r0 + nr) * W_SP)
                if eng is nc.scalar:
                    nc.scalar.copy(ob[:, sl], ps[:])
                else:
                    eng.tensor_copy(ob[:, sl], ps[:])

            # ---- sample 0: tap-outer over 7 live PSUM banks
            ps_map = {}
            for blk in range(NBLK):
                ps_map[blk] = pspool.tile(
                    [COUT, NT], f32, tag="ps", name=f"ps0_{blk}"
                )

            def pad_pe(n):
                for _ in range(n):
                    nc.tensor.matmul(
                        warm_ps[:, :128], zt[:, :128], zt[:, :128],
                        start=True, stop=True,
                    )

            for t in range(NTAP):
                kh, kw = divmod(t, KW)
                chunk, loc = wm0[t]
                for blk in BLK_ORDER0:
                    nc.tensor.matmul(
                        ps_map[blk][:],
                        chunk[:, loc * COUT : (loc + 1) * COUT],
                        rhs_ap(x0t, BLK_CH0[blk], blk * RPB, RPB, kh, kw),
                        start=(t == 0),
                        stop=(t == NTAP - 1),
                        skip_group_check=True,
                    )
                    pad_pe(PADS.get((t, blk), 0))

            # drain sample 0: fp32 PSUM -> fp16 SBUF on Scalar/Vector in
            # parallel, then one large-line store for the whole sample
            ob0 = opool.tile([COUT, H * W_SP], f16, tag="ob")
            for blk in range(NBLK):
                eng = nc.scalar if blk % 2 == 0 else nc.vector
                copy_block(eng, ob0, ps_map[blk], blk * RPB, RPB)
            nc.sync.dma_start(out_d[0], ob0[:])

            # ---- sample 1: block-outer, drains incrementally with
            # batched stores (blocks 0-3, 4-5, 6, 7)
            ob1 = opool.tile([COUT, H * W_SP], f16, tag="ob")
            store_after = {3: slice(0, 32 * W_SP),
                           5: slice(32 * W_SP, 48 * W_SP),
                           6: slice(48 * W_SP, 55 * W_SP),
                           7: slice(55 * W_SP, 56 * W_SP)}
            for blk, (r0, nr, c) in enumerate(BLKS1):
                ps = pspool.tile(
                    [COUT, nr * W_SP], f32, tag="ps", name=f"ps1_{blk}"
                )
                for t in range(NTAP):
                    kh, kw = divmod(t, KW)
                    chunk, loc = wm1[t]
                    nc.tensor.matmul(
                        ps[:],
                        chunk[:, loc * COUT : (loc + 1) * COUT],
                        rhs_ap(x1t, c, r0, nr, kh, kw),
                        start=(t == 0),
                        stop=(t == NTAP - 1),
                    )
                # last block's copy AND store both on scalar: same-engine
                # ordering avoids a cross-engine semaphore hop in the tail
                last = blk == len(BLKS1) - 1
                eng = nc.scalar if (blk % 2 == 0 or last) else nc.vector
                copy_block(eng, ob1, ps, r0, nr)
                if blk in store_after:
                    sl = store_after[blk]
                    seng = nc.scalar if last else nc.sync
                    seng.dma_start(out_d[1][:, sl], ob1[:, sl])

    nc.compile()
    return nc


def _get_nc():
    global _cached_nc
    if _cached_nc is None:
        _cached_nc = _build_nc()
    return _cached_nc


def _prep_inputs(x, routing_weights, W):
    x = np.ascontiguousarray(x, dtype=np.float32)
    routing_weights = np.ascontiguousarray(routing_weights, dtype=np.float32)
    W = np.ascontiguousarray(W, dtype=np.float32)

    xpad = np.zeros((B, CIN, HP, WP), np.float16)
    xpad[:, :, 1 : H + 1, 1 : W_SP + 1] = x.reshape(B, CIN, H, W_SP)
    xpad = xpad.reshape(B, CIN, HP * WP)

    # W[e, co, ci, kh, kw] -> wt[ci, (kh, kw, e, co)], with the per-core
    # routing scalars (broadcast over partitions) prepended
    wt = np.ascontiguousarray(
        np.transpose(W, (2, 3, 4, 0, 1)).astype(np.float16)
    ).reshape(CIN, NTAP * E * COUT)

    in_maps = []
    for c in range(NCORES):
        r = routing_weights[c * SPC : (c + 1) * SPC]  # fp32 [SPC, E]
        rb16 = r.reshape(1, SPC * E).view(np.float16)  # fp32 bits as fp16 pairs
        rb = np.broadcast_to(rb16, (128, SPC * E * 2))
        in_maps.append(
            {
                "xpad": xpad[c * SPC : (c + 1) * SPC],
                "wt": np.ascontiguousarray(np.concatenate([rb, wt], axis=1)),
            }
        )
    return in_maps


def _run(in_maps, **kwargs):
    from concourse import bass_utils

    nc = _get_nc()
    res = bass_utils.run_bass_kernel_spmd(
        nc, in_maps, core_ids=list(range(NCORES)), **kwargs
    )
    out = np.concatenate(
        [res.results[c]["out"].astype(np.float32) for c in range(NCORES)],
        axis=0,
    ).reshape(B, COUT, H, W_SP)
    return out, res


def kernel(x, routing_weights, W):
    in_maps = _prep_inputs(x, routing_weights, W)
    out, _ = _run(in_maps)
    return out
